# revision 1
# baseline (speedup 1.0000x reference)
"""Multi-head attention (B=4, L=2048, d_model=1024, 16 heads) on 8 TRN2 NeuronCores.

Sharding: core c handles batch b = c//2 and head-group g = c%2 (8 heads each).
Column-parallel QKV projections, per-head attention, row-parallel out-projection;
the host sums the two partial outputs per batch and adds the output bias.

All attention matmuls are K=128/M=128 full-array via BLOCK-DIAGONAL 2-head
packing (head A of a pair on partitions 0..63, head B on 64..127): half-array
matmuls never warm the PE clock gate and run at 1.2 GHz; full-array ones
stream at 2.4 GHz.

Per-core program (SPMD — identical program, different input values):
  - x_{q,k,v} [2048,1024] f32 --cast-DMA--> DRAM scratch bf16 --xbar transpose
    DMA--> XT tiles [128,2048] bf16
  - QT [512(col),2048(tok)] bf16; KT in block-diagonal layout kt_bd where tile
    [:, pair, kh*128:+128] = blockdiag(KT_A[d,64k], KT_B[d,64k])
  - V in block-diagonal layout v_bd [:, pair, kh] = blockdiag(V_A[64k,d],
    V_B[64k,d]) (two strided copies + two partition-shifting SBUF DMAs)
  - per (pair, q-512 chunk, k-128 chunk): one [128,1024] PSUM score tile =
    two K=128 block-diag matmuls (k-halves); exp on ScalarE -> PT bf16
    (scale=1/8, no max subtraction -- scores are ~N(0,1) for these inputs);
    ctx[128,512] += blockdiag-AV matmuls; rowsums via M=2 blockdiag-ones
    matmuls into a [2,512] PSUM tile
  - normalize: reciprocal -> DRAM bounce -> partition-broadcast read (step-0
    DRAM source AP) -> one VectorE multiply -> CT pair tile [128(col),tok]
  - out-projection: outT[oc,tok] = Wo-stationary matmuls over 4 col-chunks
Output per core: [1024, 2048] f32 = (ctx @ Wo)^T for its batch/head-group.
"""

import os

import numpy as np

import concourse.bass as bass
import concourse.tile as tile
from concourse import mybir, bacc
from concourse.bass_utils import run_bass_kernel_spmd

F32 = mybir.dt.float32
BF16 = mybir.dt.bfloat16

L = 2048          # sequence length
D = 1024          # d_model
CC = 512          # columns per core (8 heads x 64)
DK = 64           # head dim
P = 128           # partitions
SCALE = 1.0 / np.sqrt(DK)


def build_attention_core(nc, tc, pools):
    sb1, xtp, ptp, ctp, outp, misc, vsp, dram = pools

    xq = nc.dram_tensor("xq", [L, D], F32, kind="ExternalInput").ap()
    xk = nc.dram_tensor("xk", [L, D], F32, kind="ExternalInput").ap()
    xv = nc.dram_tensor("xv", [L, D], F32, kind="ExternalInput").ap()
    wq = nc.dram_tensor("wq", [D, CC], F32, kind="ExternalInput").ap()
    wk = nc.dram_tensor("wk", [D, CC], F32, kind="ExternalInput").ap()
    wv = nc.dram_tensor("wv", [D, CC], F32, kind="ExternalInput").ap()
    wo = nc.dram_tensor("wo", [CC, D], F32, kind="ExternalInput").ap()
    bq = nc.dram_tensor("bq", [CC], F32, kind="ExternalInput").ap()
    bk = nc.dram_tensor("bk", [CC], F32, kind="ExternalInput").ap()
    bv = nc.dram_tensor("bv", [CC], F32, kind="ExternalInput").ap()
    out = nc.dram_tensor("out", [D, L], F32, kind="ExternalOutput").ap()

    EXP = mybir.ActivationFunctionType.Exp
    MULT = mybir.AluOpType.mult

    # ---- constants / weights ----
    wq_sb = sb1.tile([P, D // P, CC], BF16, tag="wq")
    wk_sb = sb1.tile([P, D // P, CC], BF16, tag="wk")
    wv_sb = sb1.tile([P, D // P, CC], BF16, tag="wv")
    wo_sb = sb1.tile([P, CC // P, D], BF16, tag="wo")
    nc.gpsimd.dma_start(wq_sb[:], wq.rearrange("(o p) c -> p o c", p=P))
    nc.gpsimd.dma_start(wk_sb[:], wk.rearrange("(o p) c -> p o c", p=P))
    nc.gpsimd.dma_start(wv_sb[:], wv.rearrange("(o p) c -> p o c", p=P))
    nc.gpsimd.dma_start(wo_sb[:], wo.rearrange("(o p) c -> p o c", p=P))

    bq_sb = sb1.tile([P, CC // P], F32, tag="bq")
    bk_sb = sb1.tile([P, CC // P], F32, tag="bk")
    nc.sync.dma_start(bq_sb[:], bq.rearrange("(o p) -> p o", p=P))
    nc.sync.dma_start(bk_sb[:], bk.rearrange("(o p) -> p o", p=P))
    bv_row = sb1.tile([1, CC], BF16, tag="bv")
    nc.gpsimd.dma_start(bv_row[:], bv[None, :])

    ones_row = sb1.tile([1, P], BF16, tag="ones_row")   # K=1 lhsT for V bias
    nc.vector.memset(ones_row[:], 1.0)
    ones_bd = sb1.tile([P, 2], BF16, tag="ones_bd")     # blockdiag ones for rowsums
    nc.vector.memset(ones_bd[:], 0.0)
    nc.vector.memset(ones_bd[0:DK, 0:1], 1.0)
    nc.vector.memset(ones_bd[DK:P, 1:2], 1.0)

    # ---- x load + cast + transpose ----
    def load_xt(x):
        scr = dram.tile([L, D], BF16, tag="scr")
        for rc in range(4):
            nc.gpsimd.dma_start(scr[rc * 512:(rc + 1) * 512, :],
                                x[rc * 512:(rc + 1) * 512, :])
        xts = []
        for c in range(D // P):
            t = xtp.tile([P, L], BF16, tag="xt")
            nc.sync.dma_start_transpose(t[:], scr[:, c * P:(c + 1) * P])
            xts.append(t)
        return xts

    qt_sb = sb1.tile([P, 4, L], BF16, tag="qt")     # [col-in-pair, pair, tok]
    # kt block-diag: [:, pair, kh*128 + m]; rows<64 & m<64 -> KT_A[d, kh*64+m],
    # rows>=64 & m>=64 -> KT_B[d, kh*64+m-64], else 0
    kt_bd = sb1.tile([P, 4, 2 * L], BF16, tag="ktbd")
    nc.vector.memset(kt_bd[:], 0.0)
    # v block-diag, kh split as (parity j, tok-chunk tn): [:, pair, j, tn, m];
    # rows<64 & m<64 -> V[kh*64+r, pc+m] (head A block),
    # rows>=64 & m>=64 -> V[kh*64+(r-64), pc+64+(m-64)] (head B block), else 0
    v_bd = sb1.tile([P, 4, 2, L // P, P], BF16, tag="vbd")
    nc.vector.memset(v_bd[:], 0.0)

    # QT projection (transposed output [col, tok])
    ps_cm = tc.tile_pool(name="ps_proj", bufs=2, space="PSUM")
    ps = ps_cm.__enter__()
    xts = load_xt(xq)
    for p in range(4):
        for tn in range(4):
            ps_t = ps.tile([P, 1024], F32, tag="sc")
            acc = ps_t[:, 0:512]
            for c in range(D // P):
                nc.tensor.matmul(acc, wq_sb[:, c, p * P:(p + 1) * P],
                                 xts[c][:, tn * 512:(tn + 1) * 512],
                                 start=(c == 0), stop=(c == D // P - 1))
            nc.vector.tensor_scalar_add(qt_sb[:, p, tn * 512:(tn + 1) * 512],
                                        acc, bq_sb[:, p:p + 1])

    # KT projection straight into block-diagonal layout
    xts = load_xt(xk)
    kt_v = kt_bd.rearrange("p t (h m) -> p t h m", m=P)   # [128, 4, 32, 128]
    for p in range(4):
        for tn in range(4):
            ps_t = ps.tile([P, 1024], F32, tag="sc")
            acc = ps_t[:, 0:512]
            for c in range(D // P):
                nc.tensor.matmul(acc, wk_sb[:, c, p * P:(p + 1) * P],
                                 xts[c][:, tn * 512:(tn + 1) * 512],
                                 start=(c == 0), stop=(c == D // P - 1))
            hs = slice(tn * 8, (tn + 1) * 8)   # 8 k-halves per 512-token chunk
            acc_v = acc.rearrange("p (h m) -> p h m", m=DK)
            nc.vector.tensor_scalar_add(kt_v[0:DK, p, hs, 0:DK],
                                        acc_v[0:DK], bk_sb[0:DK, p:p + 1])
            nc.vector.tensor_scalar_add(kt_v[DK:P, p, hs, DK:P],
                                        acc_v[DK:P], bk_sb[DK:P, p:p + 1])

    # V projection; psum tile tn covers k-halves (2tn: rows 0:64, 2tn+1: rows
    # 64:128). Same-partition blocks go straight to v_bd via VectorE; the two
    # partition-shifted blocks go through a staging tile + SBUF->SBUF DMA.
    xts = load_xt(xv)
    v_stage = vsp.tile([P, L // P, 4, DK], BF16)  # rows<64: B-even, >=64: A-odd
    for tn in range(L // P):
        ps_t = ps.tile([P, 1024], F32, tag="sc")
        acc = ps_t[:, 0:512]
        for c in range(D // P):
            nc.tensor.matmul(acc, xts[c][:, tn * P:(tn + 1) * P], wv_sb[:, c, :],
                             start=(c == 0), stop=False)
        nc.tensor.matmul(acc, ones_row[:, 0:P], bv_row[:], start=False, stop=True)
        av = acc.rearrange("p (t h m) -> p t h m", h=2, m=DK)  # [128, 4, 2, 64]
        nc.vector.tensor_copy(v_bd[0:DK, :, 0, tn, 0:DK], av[0:DK, :, 0, :])
        nc.vector.tensor_copy(v_bd[DK:P, :, 1, tn, DK:P], av[DK:P, :, 1, :])
        nc.vector.tensor_copy(v_stage[0:DK, tn, :, :], av[0:DK, :, 1, :])
        nc.vector.tensor_copy(v_stage[DK:P, tn, :, :], av[DK:P, :, 0, :])
    for t in range(4):
        # B blocks of even k-halves: from psum rows 0:64 -> partitions 64:128
        nc.sync.dma_start(v_bd[DK:P, t, 0, :, DK:P], v_stage[0:DK, :, t, :])
        # A blocks of odd k-halves: from psum rows 64:128 -> partitions 0:64
        nc.sync.dma_start(v_bd[0:DK, t, 1, :, 0:DK], v_stage[DK:P, :, t, :])

    ps_cm.__exit__(None, None, None)

    # ---- attention ----
    psa_cm = tc.tile_pool(name="ps_attn", bufs=4, space="PSUM")
    psa = psa_cm.__enter__()
    psc_cm = tc.tile_pool(name="ps_ctx", bufs=2, space="PSUM")
    psc = psc_cm.__enter__()
    ct_sb = ctp.tile([P, 4, L], BF16, tag="ct")   # [col-in-pair, pair, tok]
    for p in range(4):
        for qh in range(4):
            qs = slice(qh * 512, (qh + 1) * 512)
            ctx = psc.tile([P, 512], F32, tag="ctx")
            rs = psc.tile([2, 512], F32, tag="rs")
            for kh in range(32):
                sc = psa.tile([P, 512], F32, tag="sca")
                pt = ptp.tile([P, 512], BF16, tag="pt")
                st, sp = kh == 0, kh == 31
                nc.tensor.matmul(sc[:], kt_bd[:, p, kh * P:(kh + 1) * P],
                                 qt_sb[:, p, qs], start=True, stop=True)
                nc.scalar.activation(pt[:], sc[:], EXP, scale=SCALE)
                nc.tensor.matmul(ctx[:], v_bd[:, p, kh % 2, kh // 2, :],
                                 pt[:], start=st, stop=sp)
                nc.tensor.matmul(rs[:], ones_bd[:], pt[:], start=st, stop=sp)
            # normalize: reciprocal of rowsums, broadcast across partitions
            # via a DRAM bounce (step-0 partition source APs are DRAM-only)
            rec = misc.tile([2, 512], F32, tag="rec")
            nc.vector.reciprocal(rec[:], rs[:])
            rec_d = dram.tile([2, 512], F32, tag="recd")
            nc.sync.dma_start(rec_d[:], rec[:])
            rb = misc.tile([P, 512], F32, tag="rb")
            for half, row in ((slice(0, DK), 0), (slice(DK, P), 1)):
                src = bass.AP(tensor=rec_d.tensor, offset=rec_d.offset + row * 512,
                              ap=[[0, DK], [1, 512]])
                nc.sync.dma_start(rb[half, :], src)
            nc.vector.tensor_tensor(ct_sb[:, p, qs], ctx[:], rb[:], MULT)

    psc_cm.__exit__(None, None, None)
    psa_cm.__exit__(None, None, None)

    # ---- out projection: outT[oc, tok] ----
    pso_cm = tc.tile_pool(name="ps_out", bufs=2, space="PSUM")
    pso = pso_cm.__enter__()
    for oc in range(D // P):
        o_sb = outp.tile([P, L], F32, tag="o")
        for tn in range(4):
            ps_t = pso.tile([P, 1024], F32, tag="sc")
            acc = ps_t[:, 0:512]
            for p in range(4):
                nc.tensor.matmul(acc, wo_sb[:, p, oc * P:(oc + 1) * P],
                                 ct_sb[:, p, tn * 512:(tn + 1) * 512],
                                 start=(p == 0), stop=(p == 3))
            nc.vector.tensor_copy(o_sb[:, tn * 512:(tn + 1) * 512], acc)
        nc.sync.dma_start(out[oc * P:(oc + 1) * P, :], o_sb[:])
    pso_cm.__exit__(None, None, None)


def build_bass():
    nc = bacc.Bacc("TRN2", num_devices=8, debug=False)
    with tile.TileContext(nc) as tc:
        with (
            tc.tile_pool(name="sb1", bufs=1) as sb1,
            tc.tile_pool(name="xtp", bufs=8) as xtp,
            tc.tile_pool(name="ptp", bufs=8) as ptp,
            tc.tile_pool(name="ctp", bufs=1) as ctp,
            tc.tile_pool(name="outp", bufs=1) as outp,
            tc.tile_pool(name="misc", bufs=2) as misc,
            tc.tile_pool(name="vsp", bufs=1) as vsp,
            tc.tile_pool(name="dram", bufs=3, space="DRAM") as dram,
        ):
            build_attention_core(nc, tc,
                                 (sb1, xtp, ptp, ctp, outp, misc, vsp, dram))
    nc.compile()
    return nc


_CACHE = {}


def _get_nc():
    if "nc" not in _CACHE:
        _CACHE["nc"] = build_bass()
    return _CACHE["nc"]


def make_in_maps(query, key, value, Wq, bq, Wk, bk, Wv, bv, Wo):
    f = np.ascontiguousarray
    in_maps = []
    for c in range(8):
        b, g = c // 2, c % 2
        cs = slice(g * CC, (g + 1) * CC)
        in_maps.append({
            "xq": f(query[b], dtype=np.float32),
            "xk": f(key[b], dtype=np.float32),
            "xv": f(value[b], dtype=np.float32),
            "wq": f(Wq[:, cs], dtype=np.float32),
            "wk": f(Wk[:, cs], dtype=np.float32),
            "wv": f(Wv[:, cs], dtype=np.float32),
            "wo": f(Wo[cs, :], dtype=np.float32),
            "bq": f(bq[cs], dtype=np.float32),
            "bk": f(bk[cs], dtype=np.float32),
            "bv": f(bv[cs], dtype=np.float32),
        })
    return in_maps


def kernel(query, key, value, Wq, bq, Wk, bk, Wv, bv, Wo, bo, **run_kwargs):
    query, key, value = np.asarray(query), np.asarray(key), np.asarray(value)
    Wq, Wk, Wv, Wo = np.asarray(Wq), np.asarray(Wk), np.asarray(Wv), np.asarray(Wo)
    bq, bk, bv, bo = np.asarray(bq), np.asarray(bk), np.asarray(bv), np.asarray(bo)
    nc = _get_nc()
    in_maps = make_in_maps(query, key, value, Wq, bq, Wk, bk, Wv, bv, Wo)
    res = run_bass_kernel_spmd(nc, in_maps, core_ids=list(range(8)), **run_kwargs)
    B = query.shape[0]
    out = np.empty((B, L, D), np.float32)
    for b in range(B):
        acc = res.results[2 * b]["out"].T + res.results[2 * b + 1]["out"].T
        out[b] = acc + bo[None, :].astype(np.float32)
    if run_kwargs:
        kernel.last_results = res
    return out



# revision 16
# speedup vs baseline: 1.3677x; 1.3677x over previous
"""Multi-head attention (B=4, L=2048, d_model=1024, 16 heads) on 8 TRN2 NeuronCores.

Sharding: core c handles batch b = c//2 and head-group g = c%2 (8 heads each).
Column-parallel QKV projections, per-head attention, row-parallel out-projection;
the host sums the two partial outputs per batch and adds the output bias.

v2 structure (vs block-diagonal v1):
  - Scores per head via K=64 matmuls at tile_position (0,0)/(64,0): adjacent
    A/B-head matmuls occupy disjoint row-groups and run concurrently, so the
    score phase streams fully dense (no structural zeros) -- 2x fewer PE
    cycles than block-diag packing.
  - Softmax denominators: VectorE accumulates S_h = sum_kc exp-tiles, then one
    [128,64]-of-ones matmul per head reduces partitions AND broadcasts the
    rowsum across 64 partitions; reciprocal+normalize are plain [128,512]
    VectorE ops.  No per-chunk `ones` matmuls (saves 1/3 of attention PE
    streaming), no DRAM bounce.
  - exp batched as [128,1536] ScalarE activations over 3-bank PSUM score
    tiles (amortizes the ~352-cycle ACT instruction overhead).
  - ctx pair accumulates in ONE psum bank: head-A group starts (bank-wide
    has_written clear), head-B group rides with start=False (its region's
    bits are clear after A's clear, so first write overwrites).
  - x pipeline: whole-x cast DMA (gpsimd software DGE, f32->bf16 DRAM
    bounce) then xbar transposes, ALL on the sync queue: two DMA_TRANSPOSEs
    in flight on different hwdge queues corrupt each other (shared xbar).
"""

import numpy as np

import concourse.bass as bass
import concourse.tile as tile
from concourse import mybir, bacc
from concourse.bass_utils import run_bass_kernel_spmd

F32 = mybir.dt.float32
BF16 = mybir.dt.bfloat16

L = 2048          # sequence length
D = 1024          # d_model
CC = 512          # columns per core (8 heads x 64)
DK = 64           # head dim
P = 128           # partitions
NP = 4            # head pairs per core
SCALE = 1.0 / np.sqrt(DK)
NKC = L // P      # 16 key chunks of 128 keys
NSUB = 2 * NKC    # 32 (head, kc) subs per (pair, q-chunk)


def build_attention_core(nc, tc, pools):
    sb1, xtp, ptp, osp, misc, dram, big, ctxp = pools

    xq = nc.dram_tensor("xq", [L, D], F32, kind="ExternalInput").ap()
    xk = nc.dram_tensor("xk", [L, D], F32, kind="ExternalInput").ap()
    xv = nc.dram_tensor("xv", [L, D], F32, kind="ExternalInput").ap()
    wq = nc.dram_tensor("wq", [D, CC], F32, kind="ExternalInput").ap()
    wk = nc.dram_tensor("wk", [D, CC], F32, kind="ExternalInput").ap()
    wv = nc.dram_tensor("wv", [D, CC], F32, kind="ExternalInput").ap()
    wo = nc.dram_tensor("wo", [CC, D], F32, kind="ExternalInput").ap()
    bq = nc.dram_tensor("bq", [CC], F32, kind="ExternalInput").ap()
    bk = nc.dram_tensor("bk", [CC], F32, kind="ExternalInput").ap()
    bv = nc.dram_tensor("bv", [CC], F32, kind="ExternalInput").ap()
    out = nc.dram_tensor("out", [D, L], F32, kind="ExternalOutput").ap()

    EXP = mybir.ActivationFunctionType.Exp
    MULT = mybir.AluOpType.mult
    ADD = mybir.AluOpType.add

    hwdge = [nc.sync, nc.scalar]

    # ---- weights: f32 hardware DMA + VectorE cast to bf16 ----
    wq_sb = sb1.tile([P, D // P, CC], BF16, tag="wq")
    wk_sb = sb1.tile([P, D // P, CC], BF16, tag="wk")
    wv_sb = sb1.tile([P, D // P, CC], BF16, tag="wv")
    wo_sb = sb1.tile([P, CC // P, D], BF16, tag="wo")

    bq_sb = sb1.tile([P, NP], F32, tag="bq")
    bk_sb = sb1.tile([P, NP], F32, tag="bk")
    nc.sync.dma_start(bq_sb[:], bq.rearrange("(o p) -> p o", p=P))
    nc.sync.dma_start(bk_sb[:], bk.rearrange("(o p) -> p o", p=P))
    bv_row = sb1.tile([1, CC], BF16, tag="bv")
    bv_f32 = sb1.tile([1, CC], F32, tag="bvf")
    nc.sync.dma_start(bv_f32[:], bv[None, :])
    nc.vector.tensor_copy(bv_row[:], bv_f32[:])

    ones_row = sb1.tile([1, P], BF16, tag="ones_row")   # K=1 lhsT for V bias
    nc.vector.memset(ones_row[:], 1.0)
    ones64 = sb1.tile([P, DK], BF16, tag="ones64")      # rowsum+broadcast lhsT
    nc.vector.memset(ones64[:], 1.0)

    # ---- x pipeline: chunked cast (sw DGE) + chunked xbar transposes ----
    NC_ = 4  # 512-token chunks
    tr_i = [0]

    def load_x(x):
        """Issue cast+transpose DMAs; return list of 8 xt tiles.

        One whole-x cast DMA, whole-column transposes: chunked variants
        raced the software-DGE cast completion on hardware (kt chunk-0
        corruption), so keep the consumer waiting on the full cast."""
        scr = dram.tile([L, D], BF16, tag="scr", bufs=1)
        nc.gpsimd.dma_start(scr[:], x)
        xts = [xtp.tile([P, L], BF16, tag="xt", name=f"xt{i}")
               for i in range(D // P)]
        for c in range(D // P):
            # all transposes on the sync queue: two DMA_TRANSPOSEs in
            # flight on different hwdge queues corrupt each other (shared
            # xbar state) -- observed as scattered garbage in xt tiles.
            nc.sync.dma_start_transpose(xts[c][:], scr[:, c * P:(c + 1) * P])
        return xts

    qt_sb = sb1.tile([P, NP, L], BF16, tag="qt")   # [col-in-pair, pair, tok]
    kt_sb = sb1.tile([P, NP, L], BF16, tag="kt")   # [dk(A:0:64,B:64:128), pair, key]
    v_sb = sb1.tile([P, NKC, 8, DK], BF16, tag="v")  # [key, kc, head, dim]
    ct_sb = sb1.tile([P, NP, L], BF16, tag="ct")   # [col-in-pair, pair, tok]

    # K projection (p-major so pair 0 completes first)
    xts_k = load_x(xk)
    nc.gpsimd.dma_start(wk_sb[:], wk.rearrange("(o p) c -> p o c", p=P))
    for p in range(NP):
        for tn in range(NC_):
            acc = big.tile([P, 512], F32, tag="sc")
            for c in range(D // P):
                nc.tensor.matmul(acc[:], wk_sb[:, c, p * P:(p + 1) * P],
                                 xts_k[c][:, tn * 512:(tn + 1) * 512],
                                 start=(c == 0), stop=(c == D // P - 1))
            nc.vector.tensor_scalar_add(kt_sb[:, p, tn * 512:(tn + 1) * 512],
                                        acc[:], bk_sb[:, p:p + 1])

    # V projection (tn-major; psum [tok, cols]), straight into v_sb
    xts_v = load_x(xv)
    nc.gpsimd.dma_start(wv_sb[:], wv.rearrange("(o p) c -> p o c", p=P))
    for tn in range(L // P):
        acc = big.tile([P, 512], F32, tag="sc")
        for c in range(D // P):
            nc.tensor.matmul(acc[:], xts_v[c][:, tn * P:(tn + 1) * P],
                             wv_sb[:, c, :], start=(c == 0), stop=False)
        nc.tensor.matmul(acc[:], ones_row[:], bv_row[:], start=False, stop=True)
        av = acc.rearrange("p (h m) -> p h m", m=DK)   # [128, 8, 64]
        nc.vector.tensor_copy(v_sb[:, tn, :, :], av)

    # Q projection (tn-major so q-chunk 0 completes first)
    xts_q = load_x(xq)
    nc.gpsimd.dma_start(wq_sb[:], wq.rearrange("(o p) c -> p o c", p=P))
    nc.gpsimd.dma_start(wo_sb[:], wo.rearrange("(o p) c -> p o c", p=P))
    for tn in range(NC_):
        for p in range(NP):
            acc = big.tile([P, 512], F32, tag="sc")
            for c in range(D // P):
                nc.tensor.matmul(acc[:], wq_sb[:, c, p * P:(p + 1) * P],
                                 xts_q[c][:, tn * 512:(tn + 1) * 512],
                                 start=(c == 0), stop=(c == D // P - 1))
            nc.vector.tensor_scalar_add(qt_sb[:, p, tn * 512:(tn + 1) * 512],
                                        acc[:], bq_sb[:, p:p + 1])

    # ---- attention ----
    for qh in range(4):
        qs = slice(qh * 512, (qh + 1) * 512)
        for p in range(NP):
            ctx = ctxp.tile([P, 512], F32, tag="ctx")
            # S[:, h, :] accumulates sum_kc exp-tiles for head h in bf16
            # (2x DVE mode; per-partition rounding errors average out in the
            # 128-partition ones64 reduction below).
            S = misc.tile([P, 2, 512], BF16, tag="S", bufs=4)
            s0 = 0
            while s0 < NSUB:
                ns = min(3, NSUB - s0)
                sc = big.tile([P, ns, 512], F32, tag="sc")
                for i in range(ns):
                    h, kc = (s0 + i) % 2, (s0 + i) // 2
                    hs = slice(DK * h, DK * (h + 1))
                    nc.tensor.matmul(sc[:, i, :],
                                     kt_sb[hs, p, kc * P:(kc + 1) * P],
                                     qt_sb[hs, p, qs], start=True, stop=True)
                pt = ptp.tile([P, ns, 512], BF16, tag="pt")
                nc.scalar.activation(pt[:], sc[:], EXP, scale=SCALE)
                for i in range(ns):
                    s = s0 + i
                    h, kc = s % 2, s // 2
                    nc.tensor.matmul(ctx[DK * h:DK * (h + 1), :],
                                     v_sb[:, kc, 2 * p + h, :], pt[:, i, :],
                                     start=(kc == 0),
                                     stop=(kc == NKC - 1),
                                     skip_group_check=(h == 1))
                # S accumulation: one paired (A,B) add + one single add per
                # tile; the pair starts at the even-s slot.
                j = 0 if s0 % 2 == 0 else 1
                if s0 == 0:
                    nc.vector.tensor_copy(S[:, 0:2, :], pt[:, 0:2, :])
                else:
                    nc.vector.tensor_tensor(S[:, 0:2, :], S[:, 0:2, :],
                                            pt[:, j:j + 2, :], ADD)
                if ns == 3:
                    k = 2 if j == 0 else 0
                    hk = (s0 + k) % 2
                    nc.vector.tensor_tensor(S[:, hk, :], S[:, hk, :],
                                            pt[:, k, :], ADD)
                s0 += ns
            # rowsum (partition reduce) + broadcast via all-ones matmul
            rb = ctxp.tile([P, 512], F32, tag="ctx")
            nc.tensor.matmul(rb[0:DK, :], ones64[:], S[:, 0, :],
                             start=True, stop=True)
            nc.tensor.matmul(rb[DK:P, :], ones64[:], S[:, 1, :],
                             start=True, stop=True, skip_group_check=True)
            rec = misc.tile([P, 512], F32, tag="rec", bufs=2)
            nc.vector.reciprocal_approx_fast(rec[:], rb[:])
            nc.vector.tensor_tensor(ct_sb[:, p, qs], ctx[:], rec[:], MULT)

    # ---- out projection: outT[oc, tok] ----
    for oc in range(D // P):
        for tn in range(NC_):
            acc = big.tile([P, 512], F32, tag="sc")
            for p in range(NP):
                nc.tensor.matmul(acc[:], wo_sb[:, p, oc * P:(oc + 1) * P],
                                 ct_sb[:, p, tn * 512:(tn + 1) * 512],
                                 start=(p == 0), stop=(p == NP - 1))
            o_sb = osp.tile([P, 512], F32, tag="o")
            if (oc * NC_ + tn) % 2 == 0:
                nc.vector.tensor_copy(o_sb[:], acc[:])
            else:
                nc.scalar.copy(o_sb[:], acc[:])
            nc.sync.dma_start(
                out[oc * P:(oc + 1) * P, tn * 512:(tn + 1) * 512], o_sb[:])


def build_bass():
    nc = bacc.Bacc("TRN2", num_devices=8, debug=False)
    with tile.TileContext(nc) as tc:
        with (
            tc.tile_pool(name="sb1", bufs=1) as sb1,
            tc.tile_pool(name="xtp", bufs=16) as xtp,
            tc.tile_pool(name="ptp", bufs=4) as ptp,
            tc.tile_pool(name="osp", bufs=2) as osp,
            tc.tile_pool(name="misc", bufs=1) as misc,
            tc.tile_pool(name="dram", bufs=1, space="DRAM") as dram,
            tc.tile_pool(name="big", bufs=2, space="PSUM") as big,
            tc.tile_pool(name="ctxp", bufs=2, space="PSUM") as ctxp,
        ):
            build_attention_core(
                nc, tc, (sb1, xtp, ptp, osp, misc, dram, big, ctxp))
    nc.compile()
    return nc


_CACHE = {}


def _get_nc():
    if "nc" not in _CACHE:
        _CACHE["nc"] = build_bass()
    return _CACHE["nc"]


def make_in_maps(query, key, value, Wq, bq, Wk, bk, Wv, bv, Wo):
    f = np.ascontiguousarray
    in_maps = []
    for c in range(8):
        b, g = c // 2, c % 2
        cs = slice(g * CC, (g + 1) * CC)
        in_maps.append({
            "xq": f(query[b], dtype=np.float32),
            "xk": f(key[b], dtype=np.float32),
            "xv": f(value[b], dtype=np.float32),
            "wq": f(Wq[:, cs], dtype=np.float32),
            "wk": f(Wk[:, cs], dtype=np.float32),
            "wv": f(Wv[:, cs], dtype=np.float32),
            "wo": f(Wo[cs, :], dtype=np.float32),
            "bq": f(bq[cs], dtype=np.float32),
            "bk": f(bk[cs], dtype=np.float32),
            "bv": f(bv[cs], dtype=np.float32),
        })
    return in_maps


def kernel(query, key, value, Wq, bq, Wk, bk, Wv, bv, Wo, bo, **run_kwargs):
    query, key, value = np.asarray(query), np.asarray(key), np.asarray(value)
    Wq, Wk, Wv, Wo = np.asarray(Wq), np.asarray(Wk), np.asarray(Wv), np.asarray(Wo)
    bq, bk, bv, bo = np.asarray(bq), np.asarray(bk), np.asarray(bv), np.asarray(bo)
    nc = _get_nc()
    in_maps = make_in_maps(query, key, value, Wq, bq, Wk, bk, Wv, bv, Wo)
    res = run_bass_kernel_spmd(nc, in_maps, core_ids=list(range(8)), **run_kwargs)
    B = query.shape[0]
    out = np.empty((B, L, D), np.float32)
    for b in range(B):
        acc = res.results[2 * b]["out"].T + res.results[2 * b + 1]["out"].T
        out[b] = acc + bo[None, :].astype(np.float32)
    if run_kwargs:
        kernel.last_results = res
    return out


# revision 17
# speedup vs baseline: 1.4911x; 1.0903x over previous
"""Multi-head attention (B=4, L=2048, d_model=1024, 16 heads) on 8 TRN2 NeuronCores.

Sharding: core c handles batch b = c//2 and head-group g = c%2 (8 heads each).
Column-parallel QKV projections, per-head attention, row-parallel out-projection;
the host sums the two partial outputs per batch and adds the output bias.

v3 structure:
  - Scores per head via K=64 matmuls at tile_position (0,0)/(64,0): adjacent
    A/B-head matmuls occupy disjoint row-groups and run concurrently, so the
    score phase streams fully dense (no structural zeros) -- 2x fewer PE
    cycles than block-diag packing.
  - Softmax denominators: VectorE accumulates S[:,h,:] = sum_kc exp-tiles in
    bf16 (2x DVE mode; per-partition rounding errors average out in the
    128-partition reduction), then one [128,64]-of-ones matmul per head
    reduces partitions AND broadcasts the rowsum; reciprocal_approx_fast +
    one multiply normalize.  No per-chunk `ones` matmuls, no DRAM bounce.
  - exp batched as [128,1536] ScalarE activations over 3-bank PSUM score
    tiles (amortizes the ~352-cycle ACT instruction overhead).
  - ctx pair accumulates in ONE psum bank: per-head accumulation groups on
    disjoint partition ranges (the start=True pending-zero marking is
    per-partition-range x bank).
  - x pipeline: per-x cast DMA (gpsimd software DGE, f32->bf16 DRAM bounce;
    3 in flight -- per-queue FIFO drains them in issue order) then xbar
    transposes ALL on the sync queue: two DMA_TRANSPOSEs in flight on
    different hwdge queues corrupt each other (shared xbar state).
  - load/compute order xk -> xq -> xv, projections K -> Q -> V: attention
    starts once K + the first Q chunk are projected and consumes V chunks
    as the V projection emits them.
  - out-projection in 2-wide psum groups with batched [128,1024] copies
    (DVE/ACT alternating) and one 512KB DMA per group.
"""

import numpy as np

import concourse.bass as bass
import concourse.tile as tile
from concourse import mybir, bacc
from concourse.bass_utils import run_bass_kernel_spmd

F32 = mybir.dt.float32
BF16 = mybir.dt.bfloat16

L = 2048          # sequence length
D = 1024          # d_model
CC = 512          # columns per core (8 heads x 64)
DK = 64           # head dim
P = 128           # partitions
NP = 4            # head pairs per core
SCALE = 1.0 / np.sqrt(DK)
NKC = L // P      # 16 key chunks of 128 keys
NSUB = 2 * NKC    # 32 (head, kc) subs per (pair, q-chunk)
NC_ = 4           # 512-token chunks


def build_attention_core(nc, tc, pools):
    sb1, xtp, ptp, osp, misc, dram, big, ctxp = pools

    xq = nc.dram_tensor("xq", [L, D], F32, kind="ExternalInput").ap()
    xk = nc.dram_tensor("xk", [L, D], F32, kind="ExternalInput").ap()
    xv = nc.dram_tensor("xv", [L, D], F32, kind="ExternalInput").ap()
    wq = nc.dram_tensor("wq", [D, CC], F32, kind="ExternalInput").ap()
    wk = nc.dram_tensor("wk", [D, CC], F32, kind="ExternalInput").ap()
    wv = nc.dram_tensor("wv", [D, CC], F32, kind="ExternalInput").ap()
    wo = nc.dram_tensor("wo", [CC, D], F32, kind="ExternalInput").ap()
    bq = nc.dram_tensor("bq", [CC], F32, kind="ExternalInput").ap()
    bk = nc.dram_tensor("bk", [CC], F32, kind="ExternalInput").ap()
    bv = nc.dram_tensor("bv", [CC], F32, kind="ExternalInput").ap()
    out = nc.dram_tensor("out", [D, L], F32, kind="ExternalOutput").ap()

    EXP = mybir.ActivationFunctionType.Exp
    MULT = mybir.AluOpType.mult
    ADD = mybir.AluOpType.add

    wq_sb = sb1.tile([P, D // P, CC], BF16, tag="wq")
    wk_sb = sb1.tile([P, D // P, CC], BF16, tag="wk")
    wv_sb = sb1.tile([P, D // P, CC], BF16, tag="wv")
    wo_sb = sb1.tile([P, CC // P, D], BF16, tag="wo")

    bq_sb = sb1.tile([P, NP], F32, tag="bq")
    bk_sb = sb1.tile([P, NP], F32, tag="bk")
    nc.sync.dma_start(bq_sb[:], bq.rearrange("(o p) -> p o", p=P))
    nc.sync.dma_start(bk_sb[:], bk.rearrange("(o p) -> p o", p=P))
    bv_row = sb1.tile([1, CC], BF16, tag="bv")
    bv_f32 = sb1.tile([1, CC], F32, tag="bvf")
    nc.sync.dma_start(bv_f32[:], bv[None, :])
    nc.vector.tensor_copy(bv_row[:], bv_f32[:])

    ones_row = sb1.tile([1, P], BF16, tag="ones_row")   # K=1 lhsT for V bias
    nc.vector.memset(ones_row[:], 1.0)
    ones64 = sb1.tile([P, DK], BF16, tag="ones64")      # rowsum+broadcast lhsT
    nc.vector.memset(ones64[:], 1.0)

    def load_x(x):
        """Cast (sw DGE) + transposes (sync-queue only; see docstring)."""
        scr = dram.tile([L, D], BF16, tag="scr", bufs=3)
        nc.gpsimd.dma_start(scr[:], x)
        xts = [xtp.tile([P, L], BF16, tag="xt", name=f"xt{i}")
               for i in range(D // P)]
        for c in range(D // P):
            nc.sync.dma_start_transpose(xts[c][:], scr[:, c * P:(c + 1) * P])
        return xts

    qt_sb = sb1.tile([P, NP, L], BF16, tag="qt")   # [col-in-pair, pair, tok]
    kt_sb = sb1.tile([P, NP, L], BF16, tag="kt")   # [dk(A:0:64,B:64:128), pair, key]
    v_sb = sb1.tile([P, NKC, 8, DK], BF16, tag="v")  # [key, kc, head, dim]
    ct_sb = sb1.tile([P, NP, L], BF16, tag="ct")   # [col-in-pair, pair, tok]

    # K projection (p-major so pair 0 completes first)
    xts_k = load_x(xk)
    nc.gpsimd.dma_start(wk_sb[:], wk.rearrange("(o p) c -> p o c", p=P))
    for p in range(NP):
        for tn in range(NC_):
            acc = big.tile([P, 512], F32, tag="sc")
            for c in range(D // P):
                nc.tensor.matmul(acc[:], wk_sb[:, c, p * P:(p + 1) * P],
                                 xts_k[c][:, tn * 512:(tn + 1) * 512],
                                 start=(c == 0), stop=(c == D // P - 1))
            nc.vector.tensor_scalar_add(kt_sb[:, p, tn * 512:(tn + 1) * 512],
                                        acc[:], bk_sb[:, p:p + 1])

    # Q projection (tn-major so q-chunk 0 completes first)
    xts_q = load_x(xq)
    nc.gpsimd.dma_start(wq_sb[:], wq.rearrange("(o p) c -> p o c", p=P))
    for tn in range(NC_):
        for p in range(NP):
            acc = big.tile([P, 512], F32, tag="sc")
            for c in range(D // P):
                nc.tensor.matmul(acc[:], wq_sb[:, c, p * P:(p + 1) * P],
                                 xts_q[c][:, tn * 512:(tn + 1) * 512],
                                 start=(c == 0), stop=(c == D // P - 1))
            nc.vector.tensor_scalar_add(qt_sb[:, p, tn * 512:(tn + 1) * 512],
                                        acc[:], bq_sb[:, p:p + 1])

    # V projection (tn-major; psum [tok, cols]), straight into v_sb;
    # attention consumes v_sb[:, kc] chunks as they land.
    xts_v = load_x(xv)
    nc.gpsimd.dma_start(wv_sb[:], wv.rearrange("(o p) c -> p o c", p=P))
    nc.gpsimd.dma_start(wo_sb[:], wo.rearrange("(o p) c -> p o c", p=P))
    for tn in range(L // P):
        acc = big.tile([P, 512], F32, tag="sc")
        for c in range(D // P):
            nc.tensor.matmul(acc[:], xts_v[c][:, tn * P:(tn + 1) * P],
                             wv_sb[:, c, :], start=(c == 0), stop=False)
        nc.tensor.matmul(acc[:], ones_row[:], bv_row[:], start=False, stop=True)
        av = acc.rearrange("p (h m) -> p h m", m=DK)   # [128, 8, 64]
        nc.vector.tensor_copy(v_sb[:, tn, :, :], av)

    # ---- attention ----
    for qh in range(4):
        qs = slice(qh * 512, (qh + 1) * 512)
        for p in range(NP):
            ctx = ctxp.tile([P, 512], F32, tag="ctx")
            # S[:, h, :] accumulates sum_kc exp-tiles for head h in bf16.
            S = misc.tile([P, 2, 512], BF16, tag="S", bufs=4)
            s0 = 0
            while s0 < NSUB:
                ns = min(3, NSUB - s0)
                sc = big.tile([P, ns, 512], F32, tag="sc")
                for i in range(ns):
                    h, kc = (s0 + i) % 2, (s0 + i) // 2
                    hs = slice(DK * h, DK * (h + 1))
                    nc.tensor.matmul(sc[:, i, :],
                                     kt_sb[hs, p, kc * P:(kc + 1) * P],
                                     qt_sb[hs, p, qs], start=True, stop=True)
                pt = ptp.tile([P, ns, 512], BF16, tag="pt")
                nc.scalar.activation(pt[:], sc[:], EXP, scale=SCALE)
                for i in range(ns):
                    s = s0 + i
                    h, kc = s % 2, s // 2
                    nc.tensor.matmul(ctx[DK * h:DK * (h + 1), :],
                                     v_sb[:, kc, 2 * p + h, :], pt[:, i, :],
                                     start=(kc == 0),
                                     stop=(kc == NKC - 1),
                                     skip_group_check=(h == 1))
                # S accumulation: one paired (A,B) add + one single add per
                # tile; the pair starts at the even-s slot.
                j = 0 if s0 % 2 == 0 else 1
                if s0 == 0:
                    nc.vector.tensor_copy(S[:, 0:2, :], pt[:, 0:2, :])
                else:
                    nc.vector.tensor_tensor(S[:, 0:2, :], S[:, 0:2, :],
                                            pt[:, j:j + 2, :], ADD)
                if ns == 3:
                    k = 2 if j == 0 else 0
                    hk = (s0 + k) % 2
                    nc.vector.tensor_tensor(S[:, hk, :], S[:, hk, :],
                                            pt[:, k, :], ADD)
                s0 += ns
            # rowsum (partition reduce) + broadcast via all-ones matmul
            rb = ctxp.tile([P, 512], F32, tag="ctx")
            nc.tensor.matmul(rb[0:DK, :], ones64[:], S[:, 0, :],
                             start=True, stop=True)
            nc.tensor.matmul(rb[DK:P, :], ones64[:], S[:, 1, :],
                             start=True, stop=True, skip_group_check=True)
            rec = misc.tile([P, 512], F32, tag="rec", bufs=2)
            nc.vector.reciprocal_approx_fast(rec[:], rb[:])
            nc.vector.tensor_tensor(ct_sb[:, p, qs], ctx[:], rec[:], MULT)

    # ---- out projection: outT[oc, tok], 2-wide psum groups ----
    for g in range(16):
        oc, tn0 = g // 2, (g % 2) * 2
        acc = big.tile([P, 2, 512], F32, tag="sc")
        for j in range(2):
            tn = tn0 + j
            for p in range(NP):
                nc.tensor.matmul(acc[:, j, :],
                                 wo_sb[:, p, oc * P:(oc + 1) * P],
                                 ct_sb[:, p, tn * 512:(tn + 1) * 512],
                                 start=(p == 0), stop=(p == NP - 1))
        o_sb = osp.tile([P, 2, 512], F32, tag="o")
        if g % 2 == 0:
            nc.vector.tensor_copy(o_sb[:], acc[:])
        else:
            nc.scalar.copy(o_sb[:], acc[:])
        nc.sync.dma_start(out[oc * P:(oc + 1) * P, tn0 * 512:(tn0 + 2) * 512],
                          o_sb[:])


def build_bass():
    nc = bacc.Bacc("TRN2", num_devices=8, debug=False)
    with tile.TileContext(nc) as tc:
        with (
            tc.tile_pool(name="sb1", bufs=1) as sb1,
            tc.tile_pool(name="xtp", bufs=16) as xtp,
            tc.tile_pool(name="ptp", bufs=4) as ptp,
            tc.tile_pool(name="osp", bufs=3) as osp,
            tc.tile_pool(name="misc", bufs=1) as misc,
            tc.tile_pool(name="dram", bufs=1, space="DRAM") as dram,
            tc.tile_pool(name="big", bufs=2, space="PSUM") as big,
            tc.tile_pool(name="ctxp", bufs=2, space="PSUM") as ctxp,
        ):
            build_attention_core(
                nc, tc, (sb1, xtp, ptp, osp, misc, dram, big, ctxp))
    nc.compile()
    return nc


_CACHE = {}


def _get_nc():
    if "nc" not in _CACHE:
        _CACHE["nc"] = build_bass()
    return _CACHE["nc"]


def make_in_maps(query, key, value, Wq, bq, Wk, bk, Wv, bv, Wo):
    f = np.ascontiguousarray
    in_maps = []
    for c in range(8):
        b, g = c // 2, c % 2
        cs = slice(g * CC, (g + 1) * CC)
        in_maps.append({
            "xq": f(query[b], dtype=np.float32),
            "xk": f(key[b], dtype=np.float32),
            "xv": f(value[b], dtype=np.float32),
            "wq": f(Wq[:, cs], dtype=np.float32),
            "wk": f(Wk[:, cs], dtype=np.float32),
            "wv": f(Wv[:, cs], dtype=np.float32),
            "wo": f(Wo[cs, :], dtype=np.float32),
            "bq": f(bq[cs], dtype=np.float32),
            "bk": f(bk[cs], dtype=np.float32),
            "bv": f(bv[cs], dtype=np.float32),
        })
    return in_maps


def kernel(query, key, value, Wq, bq, Wk, bk, Wv, bv, Wo, bo, **run_kwargs):
    query, key, value = np.asarray(query), np.asarray(key), np.asarray(value)
    Wq, Wk, Wv, Wo = np.asarray(Wq), np.asarray(Wk), np.asarray(Wv), np.asarray(Wo)
    bq, bk, bv, bo = np.asarray(bq), np.asarray(bk), np.asarray(bv), np.asarray(bo)
    nc = _get_nc()
    in_maps = make_in_maps(query, key, value, Wq, bq, Wk, bk, Wv, bv, Wo)
    res = run_bass_kernel_spmd(nc, in_maps, core_ids=list(range(8)), **run_kwargs)
    B = query.shape[0]
    out = np.empty((B, L, D), np.float32)
    for b in range(B):
        acc = res.results[2 * b]["out"].T + res.results[2 * b + 1]["out"].T
        out[b] = acc + bo[None, :].astype(np.float32)
    if run_kwargs:
        kernel.last_results = res
    return out
